# revision 1
# baseline (speedup 1.0000x reference)
"""Trainium2 Bass kernel for quantized Conv1D forward:
    y = x @ (w_q * scale) + bias
  x:     [4, 2048, 4096] f32
  w_q:   [4096, 16384] int32 (values in [-127, 127])
  scale: [16384] f32
  bias:  [16384] f32
  y:     [4, 2048, 16384] f32

Sharding: column-parallel over out_features across 8 cores (N=2048 each);
x replicated. Each core computes y_shard = x @ (w_q_shard * scale_shard)
+ bias_shard independently (no collectives); host concatenates shards.

Device strategy:
  - w_q is exactly representable in fp16 (|w| <= 127 < 2048), so the
    matmul runs in fp16 with the *weight* error exactly zero. scale is
    applied after the matmul (exact algebraic identity), bias added last.
  - x is split into x_hi = fp16(x), x_lo = fp16(x - x_hi); the two fp16
    matmuls accumulate into the same PSUM bank in fp32, recovering ~fp32
    precision (measured ~5e-8 rel err vs f64, below the f32 reference's
    own ~3e-7).  Set X_SPLIT=False for a single-pass fp16 x (~2e-4).
  - The fp16 weight shard [4096, 2048] stays fully resident in SBUF
    (128 KB/partition); x tiles stream through; PE runs back-to-back
    matmuls (stationary = x^T tile, moving = w rows, N=512 per PSUM bank).
"""

import numpy as np

import concourse.bass as bass
import concourse.mybir as mybir
import concourse.tile as tile
from concourse import bacc
from concourse.bass import ts
from concourse.bass_utils import run_bass_kernel_spmd

P = 128
N_CORES = 8

# numerics strategy: True = x split into fp16 hi+lo (2 matmul passes,
# ~fp32-exact); False = single fp16 pass for x (~2e-4 rel err, 2x faster)
X_SPLIT = True


def build_nc(T, K, N, x_split=X_SPLIT, n_free=512):
    """Build the per-core Bass program.

    DRAM I/O (per core):
      xh:    [TB, P, S, KB, Tt] fp16  packed x^T tiles (S=2 if split: hi,lo)
      wh:    [P, KB, N]         fp16  weight shard, k on partitions
      scale: [N] f32
      bias:  [N] f32
      y:     [T, N] f32 out
    """
    KB = K // P
    TB = T // P
    Tt = P
    NB = N // n_free
    S = 2 if x_split else 1

    nc = bacc.Bacc("TRN2", target_bir_lowering=False, debug=False)

    xh = nc.dram_tensor("xh", [TB, P, S, KB, Tt], mybir.dt.float16, kind="ExternalInput")
    wh = nc.dram_tensor("wh", [P, KB, N], mybir.dt.float16, kind="ExternalInput")
    scale_h = nc.dram_tensor("scale", [N], mybir.dt.float32, kind="ExternalInput")
    bias_h = nc.dram_tensor("bias", [N], mybir.dt.float32, kind="ExternalInput")
    y_h = nc.dram_tensor("y", [T, N], mybir.dt.float32, kind="ExternalOutput")

    xh_ap = xh.ap()
    wh_ap = wh.ap()
    y_ap = y_h.ap().rearrange("(tb p) n -> tb p n", p=P)

    def bcast_ap(ap):
        # [N] dram vector -> [P, N] with step-0 partition dim for DMA broadcast
        return bass.AP(tensor=ap.tensor, offset=ap.offset, ap=[[0, P], *ap.ap])

    with tile.TileContext(nc) as tc:
        with (
            tc.tile_pool(name="wpool", bufs=1) as wpool,
            tc.tile_pool(name="cpool", bufs=1) as cpool,
            tc.tile_pool(name="xpool", bufs=2) as xpool,
            tc.tile_pool(name="opool", bufs=2) as opool,
            tc.tile_pool(name="ppool", bufs=2 * NB, space="PSUM") as ppool,
        ):
            # resident weights: [P, KB, N] fp16
            w_res = wpool.tile([P, KB, N], mybir.dt.float16, name="w_res")
            for kb in range(KB):
                nc.sync.dma_start(out=w_res[:, kb], in_=wh_ap[:, kb])

            scale_b = cpool.tile([P, N], mybir.dt.float32, name="scale_b")
            bias_b = cpool.tile([P, N], mybir.dt.float32, name="bias_b")
            nc.sync.dma_start(out=scale_b[:], in_=bcast_ap(scale_h.ap()))
            nc.sync.dma_start(out=bias_b[:], in_=bcast_ap(bias_h.ap()))

            for tb in range(TB):
                xt = xpool.tile([P, S, KB, Tt], mybir.dt.float16, tag="xt")
                nc.sync.dma_start(out=xt[:], in_=xh_ap[tb])

                psums = [
                    ppool.tile([P, n_free], mybir.dt.float32, tag="acc", name=f"ps{nb}")
                    for nb in range(NB)
                ]
                for kb in range(KB):
                    for s in range(S):
                        for nb in range(NB):
                            nc.tensor.matmul(
                                psums[nb][:],
                                lhsT=xt[:, s, kb, :],
                                rhs=w_res[:, kb, ts(nb, n_free)],
                                start=(kb == 0 and s == 0),
                                stop=(kb == KB - 1 and s == S - 1),
                            )

                out_sb = opool.tile([P, N], mybir.dt.float32, tag="out")
                for nb in range(NB):
                    nc.vector.tensor_mul(
                        out=out_sb[:, ts(nb, n_free)],
                        in0=psums[nb][:],
                        in1=scale_b[:, ts(nb, n_free)],
                    )
                    nc.vector.tensor_add(
                        out=out_sb[:, ts(nb, n_free)],
                        in0=out_sb[:, ts(nb, n_free)],
                        in1=bias_b[:, ts(nb, n_free)],
                    )
                nc.sync.dma_start(out=y_ap[tb], in_=out_sb[:])

    nc.compile()
    return nc


def pack_x(x2d, T, K, x_split=X_SPLIT):
    """[T, K] f32 -> [TB, P, S, KB, Tt] fp16 tiles of x^T (hi[, lo])."""
    TB, KB = T // P, K // P
    x_hi = x2d.astype(np.float16)
    # [T, K] -> [TB, Tt, KB, Pk] -> [TB, Pk, KB, Tt]
    def tilev(a):
        return np.ascontiguousarray(
            a.reshape(TB, P, KB, P).transpose(0, 3, 2, 1)
        )
    if not x_split:
        return tilev(x_hi)[:, :, None, :, :]
    x_lo = (x2d - x_hi.astype(np.float32)).astype(np.float16)
    out = np.empty((TB, P, 2, KB, P), dtype=np.float16)
    out[:, :, 0] = tilev(x_hi)
    out[:, :, 1] = tilev(x_lo)
    return out


def pack_w(w_shard, K, N):
    """[K, N] int -> [P, KB, N] fp16 (exact)."""
    KB = K // P
    return np.ascontiguousarray(
        w_shard.astype(np.float16).reshape(KB, P, N).transpose(1, 0, 2)
    )


_NC_CACHE = {}


def _get_nc(T, K, N, x_split):
    key = (T, K, N, x_split)
    if key not in _NC_CACHE:
        _NC_CACHE[key] = build_nc(T, K, N, x_split=x_split)
    return _NC_CACHE[key]


def kernel(x, w_q, scale, bias):
    B, Sq, K = x.shape
    K2, D_OUT = w_q.shape
    assert K2 == K
    T = B * Sq
    N = D_OUT // N_CORES

    nc = _get_nc(T, K, N, X_SPLIT)

    xh = pack_x(np.ascontiguousarray(x.reshape(T, K)), T, K, X_SPLIT)
    in_maps = []
    for c in range(N_CORES):
        sl = slice(c * N, (c + 1) * N)
        in_maps.append(
            {
                "xh": xh,
                "wh": pack_w(w_q[:, sl], K, N),
                "scale": np.ascontiguousarray(scale[sl], dtype=np.float32),
                "bias": np.ascontiguousarray(bias[sl], dtype=np.float32),
            }
        )

    res = run_bass_kernel_spmd(nc, in_maps, core_ids=list(range(N_CORES)))
    y = np.concatenate([r["y"] for r in res.results], axis=1)
    return y.reshape(B, Sq, D_OUT)


# revision 3
# speedup vs baseline: 17.4662x; 17.4662x over previous
"""Trainium2 Bass kernel for quantized Conv1D forward:
    y = x @ (w_q * scale) + bias
  x:     [4, 2048, 4096] f32
  w_q:   [4096, 16384] int32 (values in [-127, 127])
  scale: [16384] f32
  bias:  [16384] f32
  y:     [4, 2048, 16384] f32

Sharding: column-parallel over out_features across 8 cores (N=2048 each);
x replicated. Each core computes y_shard = x @ (w_q_shard * scale_shard)
+ bias_shard independently (no collectives); host concatenates shards.

Device strategy:
  - w_q is exactly representable in fp16 (|w| <= 127 < 2048), so the
    matmul runs in fp16 with the *weight* error exactly zero. scale is
    applied after the matmul (exact algebraic identity), bias added last.
  - x is split into x_hi = fp16(x), x_lo = fp16(x - x_hi); the two fp16
    matmuls accumulate into the same PSUM bank in fp32, recovering ~fp32
    precision (measured ~5e-8 rel err vs f64, below the f32 reference's
    own ~3e-7).  Set X_SPLIT=False for a single-pass fp16 x (~2e-4).
  - The fp16 weight shard [4096, 2048] stays fully resident in SBUF
    (128 KB/partition); x tiles stream through; PE runs back-to-back
    matmuls (stationary = x^T tile, moving = w rows, N=512 per PSUM bank).
"""

import numpy as np

import concourse.bass as bass
import concourse.mybir as mybir
import concourse.tile as tile
from concourse import bacc
from concourse.bass import ts
from concourse.bass_utils import run_bass_kernel_spmd

P = 128
N_CORES = 8

# numerics strategy: True = x split into fp16 hi+lo (2 matmul passes,
# ~fp32-exact); False = single fp16 pass for x (~2e-4 rel err, 2x faster)
X_SPLIT = True


def build_nc(T, K, N, x_split=X_SPLIT, n_free=512, reps=1):
    """Build the per-core Bass program.

    DRAM I/O (per core):
      xh:    [TB, P, S, KB, Tt] fp16  packed x^T tiles (S=2 if split: hi,lo)
      wh:    [P, KB, N]         fp16  weight shard, k on partitions
      scale: [N] f32
      bias:  [N] f32
      y:     [T, N] f32 out
    """
    KB = K // P
    TB = T // P
    Tt = P
    NB = N // n_free
    S = 2 if x_split else 1

    nc = bacc.Bacc("TRN2", target_bir_lowering=False, debug=False)

    xh = nc.dram_tensor("xh", [TB, P, S, KB, Tt], mybir.dt.float16, kind="ExternalInput")
    wh = nc.dram_tensor("wh", [P, KB, N], mybir.dt.float16, kind="ExternalInput")
    scale_h = nc.dram_tensor("scale", [N], mybir.dt.float32, kind="ExternalInput")
    bias_h = nc.dram_tensor("bias", [N], mybir.dt.float32, kind="ExternalInput")
    y_h = nc.dram_tensor("y", [T, N], mybir.dt.float32, kind="ExternalOutput")

    xh_ap = xh.ap()
    wh_ap = wh.ap()
    y_ap = y_h.ap().rearrange("(tb p) n -> tb p n", p=P)

    def bcast_ap(ap):
        # [N] dram vector -> [P, N] with step-0 partition dim for DMA broadcast
        return bass.AP(tensor=ap.tensor, offset=ap.offset, ap=[[0, P], *ap.ap])

    with tile.TileContext(nc) as tc:
        with (
            tc.tile_pool(name="wpool", bufs=1) as wpool,
            tc.tile_pool(name="cpool", bufs=1) as cpool,
            tc.tile_pool(name="xpool", bufs=2) as xpool,
            tc.tile_pool(name="opool", bufs=2) as opool,
            tc.tile_pool(name="ppool", bufs=2 * NB, space="PSUM") as ppool,
        ):
            # resident weights: [P, KB, N] fp16
            w_res = wpool.tile([P, KB, N], mybir.dt.float16, name="w_res")
            for kb in range(KB):
                nc.sync.dma_start(out=w_res[:, kb], in_=wh_ap[:, kb])

            scale_b = cpool.tile([P, N], mybir.dt.float32, name="scale_b")
            bias_b = cpool.tile([P, N], mybir.dt.float32, name="bias_b")
            nc.sync.dma_start(out=scale_b[:], in_=bcast_ap(scale_h.ap()))
            nc.sync.dma_start(out=bias_b[:], in_=bcast_ap(bias_h.ap()))

            for tb in [t for _ in range(reps) for t in range(TB)]:
                xt = xpool.tile([P, S, KB, Tt], mybir.dt.float16, tag="xt")
                nc.sync.dma_start(out=xt[:], in_=xh_ap[tb])

                psums = [
                    ppool.tile([P, n_free], mybir.dt.float32, tag="acc", name=f"ps{nb}")
                    for nb in range(NB)
                ]
                for kb in range(KB):
                    for s in range(S):
                        for nb in range(NB):
                            nc.tensor.matmul(
                                psums[nb][:],
                                lhsT=xt[:, s, kb, :],
                                rhs=w_res[:, kb, ts(nb, n_free)],
                                start=(kb == 0 and s == 0),
                                stop=(kb == KB - 1 and s == S - 1),
                            )

                out_sb = opool.tile([P, N], mybir.dt.float32, tag="out")
                for nb in range(NB):
                    nc.vector.tensor_mul(
                        out=out_sb[:, ts(nb, n_free)],
                        in0=psums[nb][:],
                        in1=scale_b[:, ts(nb, n_free)],
                    )
                    nc.vector.tensor_add(
                        out=out_sb[:, ts(nb, n_free)],
                        in0=out_sb[:, ts(nb, n_free)],
                        in1=bias_b[:, ts(nb, n_free)],
                    )
                nc.sync.dma_start(out=y_ap[tb], in_=out_sb[:])

    nc.compile()
    return nc


def pack_x(x2d, T, K, x_split=X_SPLIT):
    """[T, K] f32 -> [TB, P, S, KB, Tt] fp16 tiles of x^T (hi[, lo])."""
    TB, KB = T // P, K // P
    x_hi = x2d.astype(np.float16)
    # [T, K] -> [TB, Tt, KB, Pk] -> [TB, Pk, KB, Tt]
    def tilev(a):
        return np.ascontiguousarray(
            a.reshape(TB, P, KB, P).transpose(0, 3, 2, 1)
        )
    if not x_split:
        return tilev(x_hi)[:, :, None, :, :]
    x_lo = (x2d - x_hi.astype(np.float32)).astype(np.float16)
    out = np.empty((TB, P, 2, KB, P), dtype=np.float16)
    out[:, :, 0] = tilev(x_hi)
    out[:, :, 1] = tilev(x_lo)
    return out


def pack_w(w_shard, K, N):
    """[K, N] int -> [P, KB, N] fp16 (exact)."""
    KB = K // P
    return np.ascontiguousarray(
        w_shard.astype(np.float16).reshape(KB, P, N).transpose(1, 0, 2)
    )


_NC_CACHE = {}


def _get_nc(T, K, N, x_split):
    key = (T, K, N, x_split)
    if key not in _NC_CACHE:
        _NC_CACHE[key] = build_nc(T, K, N, x_split=x_split)
    return _NC_CACHE[key]


def kernel(x, w_q, scale, bias):
    B, Sq, K = x.shape
    K2, D_OUT = w_q.shape
    assert K2 == K
    T = B * Sq
    N = D_OUT // N_CORES

    nc = _get_nc(T, K, N, X_SPLIT)

    xh = pack_x(np.ascontiguousarray(x.reshape(T, K)), T, K, X_SPLIT)
    in_maps = []
    for c in range(N_CORES):
        sl = slice(c * N, (c + 1) * N)
        in_maps.append(
            {
                "xh": xh,
                "wh": pack_w(w_q[:, sl], K, N),
                "scale": np.ascontiguousarray(scale[sl], dtype=np.float32),
                "bias": np.ascontiguousarray(bias[sl], dtype=np.float32),
            }
        )

    res = run_bass_kernel_spmd(nc, in_maps, core_ids=list(range(N_CORES)))
    y = np.concatenate([r["y"] for r in res.results], axis=1)
    return y.reshape(B, Sq, D_OUT)


# revision 18
# speedup vs baseline: 18.6445x; 1.0675x over previous
"""Trainium2 Bass kernel for quantized Conv1D forward:
    y = x @ (w_q * scale) + bias
  x:     [4, 2048, 4096] f32
  w_q:   [4096, 16384] int32 (values in [-127, 127])
  scale: [16384] f32
  bias:  [16384] f32
  y:     [4, 2048, 16384] f32

Sharding: column-parallel over out_features across 8 cores (N=2048 each);
x replicated. Each core computes y_shard = x @ (w_q_shard * scale_shard)
+ bias_shard independently (no collectives); host concatenates shards.

Device strategy:
  - w_q is exactly representable in fp16 (|w| <= 127 < 2048), so the
    matmul runs in fp16 with the *weight* error exactly zero. scale is
    applied after the matmul (exact algebraic identity), bias added last.
  - x is split into x_hi = fp16(x), x_lo = fp16(x - x_hi); the two fp16
    matmuls accumulate into the same PSUM bank in fp32, recovering ~fp32
    precision (measured ~5e-8 rel err vs f64, below the f32 reference's
    own ~3e-7).  Set X_SPLIT=False for a single-pass fp16 x (~2e-4).
  - The fp16 weight shard [4096, 2048] stays fully resident in SBUF
    (128 KB/partition); x tiles stream through; PE runs back-to-back
    matmuls (stationary = x^T tile, moving = w rows, N=512 per PSUM bank).
"""

import numpy as np

import concourse.bass as bass
import concourse.mybir as mybir
import concourse.tile as tile
from concourse import bacc
from concourse.bass import ts
from concourse.bass_utils import run_bass_kernel_spmd

P = 128
N_CORES = 8

# numerics strategy: True = x split into fp16 hi+lo (2 matmul passes,
# ~fp32-exact); False = single fp16 pass for x (~2e-4 rel err, 2x faster)
X_SPLIT = True


def build_nc(T, K, N, x_split=X_SPLIT, n_free=512, reps=1,
             x_bufs=2, o_bufs=2, p_bufs=None, x_dma_split=1, swap_loop=False,
             mm_dt="fp16", w_split=False, y_dma_split=1):
    """Build the per-core Bass program.

    DRAM I/O (per core):
      xh:    [TB, P, S, KB, Tt] fp16  packed x^T tiles (S=2 if split: hi,lo)
      wh:    [P, KB, N]         fp16  weight shard, k on partitions
      scale: [N] f32
      bias:  [N] f32
      y:     [T, N] f32 out
    """
    KB = K // P
    TB = T // P
    Tt = P
    NB = N // n_free
    S = 2 if x_split else 1
    mdt = {"fp16": mybir.dt.float16, "bf16": mybir.dt.bfloat16,
           "fp32r": mybir.dt.float32r}[mm_dt]

    nc = bacc.Bacc("TRN2", target_bir_lowering=False, debug=False)

    xh = nc.dram_tensor("xh", [TB, P, S, KB, Tt], mdt, kind="ExternalInput")
    wh = nc.dram_tensor("wh", [P, KB, N], mdt, kind="ExternalInput")
    scale_h = nc.dram_tensor("scale", [N], mybir.dt.float32, kind="ExternalInput")
    bias_h = nc.dram_tensor("bias", [N], mybir.dt.float32, kind="ExternalInput")
    y_h = nc.dram_tensor("y", [T, N], mybir.dt.float32, kind="ExternalOutput")

    xh_ap = xh.ap()
    wh_ap = wh.ap()
    y_ap = y_h.ap().rearrange("(tb p) n -> tb p n", p=P)

    def bcast_ap(ap):
        # [N] dram vector -> [P, N] with step-0 partition dim for DMA broadcast
        return bass.AP(tensor=ap.tensor, offset=ap.offset, ap=[[0, P], *ap.ap])

    with tile.TileContext(nc) as tc:
        if p_bufs is None:
            p_bufs = 2 * NB
        with (
            tc.tile_pool(name="wpool", bufs=1) as wpool,
            tc.tile_pool(name="cpool", bufs=1) as cpool,
            tc.tile_pool(name="xpool", bufs=x_bufs) as xpool,
            tc.tile_pool(name="opool", bufs=o_bufs) as opool,
            tc.tile_pool(name="ppool", bufs=p_bufs, space="PSUM") as ppool,
        ):
            # resident weights: [P, KB, N]; per-kb tiles give per-slice deps
            if w_split:
                w_tiles = []
                for kb in range(KB):
                    wt = wpool.tile([P, N], mdt, name=f"w{kb}")
                    nc.sync.dma_start(out=wt[:], in_=wh_ap[:, kb])
                    w_tiles.append(wt)
                w_rhs = lambda kb, nb: w_tiles[kb][:, ts(nb, n_free)]
            else:
                w_res = wpool.tile([P, KB, N], mdt, name="w_res")
                for kb in range(KB):
                    nc.sync.dma_start(out=w_res[:, kb], in_=wh_ap[:, kb])
                w_rhs = lambda kb, nb: w_res[:, kb, ts(nb, n_free)]

            scale_b = cpool.tile([P, N], mybir.dt.float32, name="scale_b")
            bias_b = cpool.tile([P, N], mybir.dt.float32, name="bias_b")
            nc.sync.dma_start(out=scale_b[:], in_=bcast_ap(scale_h.ap()))
            nc.sync.dma_start(out=bias_b[:], in_=bcast_ap(bias_h.ap()))

            for tb in [t for _ in range(reps) for t in range(TB)]:
                xt = xpool.tile([P, S, KB, Tt], mdt, tag="xt")
                if x_dma_split == 1:
                    nc.sync.dma_start(out=xt[:], in_=xh_ap[tb])
                else:
                    assert KB % x_dma_split == 0
                    c = KB // x_dma_split
                    for s in range(S):
                        for d in range(x_dma_split):
                            nc.sync.dma_start(
                                out=xt[:, s, ts(d, c)],
                                in_=xh_ap[tb, :, s, ts(d, c)],
                            )

                psums = [
                    ppool.tile([P, n_free], mybir.dt.float32, tag="acc", name=f"ps{nb}")
                    for nb in range(NB)
                ]
                if swap_loop:
                    mm_iter = [
                        (kb, s, nb)
                        for nb in range(NB)
                        for kb in range(KB)
                        for s in range(S)
                    ]
                else:
                    mm_iter = [
                        (kb, s, nb)
                        for kb in range(KB)
                        for s in range(S)
                        for nb in range(NB)
                    ]
                for kb, s, nb in mm_iter:
                    nc.tensor.matmul(
                        psums[nb][:],
                        lhsT=xt[:, s, kb, :],
                        rhs=w_rhs(kb, nb),
                        start=(kb == 0 and s == 0),
                        stop=(kb == KB - 1 and s == S - 1),
                    )

                out_sb = opool.tile([P, N], mybir.dt.float32, tag="out")
                for nb in range(NB):
                    nc.vector.tensor_mul(
                        out=out_sb[:, ts(nb, n_free)],
                        in0=psums[nb][:],
                        in1=scale_b[:, ts(nb, n_free)],
                    )
                    nc.vector.tensor_add(
                        out=out_sb[:, ts(nb, n_free)],
                        in0=out_sb[:, ts(nb, n_free)],
                        in1=bias_b[:, ts(nb, n_free)],
                    )
                if y_dma_split == 1:
                    nc.sync.dma_start(out=y_ap[tb], in_=out_sb[:])
                else:
                    c = N // y_dma_split
                    for d in range(y_dma_split):
                        nc.sync.dma_start(
                            out=y_ap[tb, :, ts(d, c)], in_=out_sb[:, ts(d, c)]
                        )

    nc.compile()
    return nc


def pack_x(x2d, T, K, x_split=X_SPLIT, np_dt=np.float16):
    """[T, K] f32 -> [TB, P, S, KB, Tt] tiles of x^T (hi[, lo])."""
    TB, KB = T // P, K // P
    x_hi = x2d.astype(np_dt)
    # [T, K] -> [TB, Tt, KB, Pk] -> [TB, Pk, KB, Tt]
    def tilev(a):
        return np.ascontiguousarray(
            a.reshape(TB, P, KB, P).transpose(0, 3, 2, 1)
        )
    if not x_split:
        return tilev(x_hi)[:, :, None, :, :]
    x_lo = (x2d - x_hi.astype(np.float32)).astype(np_dt)
    out = np.empty((TB, P, 2, KB, P), dtype=np_dt)
    out[:, :, 0] = tilev(x_hi)
    out[:, :, 1] = tilev(x_lo)
    return out


def pack_w(w_shard, K, N, np_dt=np.float16):
    """[K, N] int -> [P, KB, N] (exact in fp16/bf16/f32)."""
    KB = K // P
    return np.ascontiguousarray(
        w_shard.astype(np_dt).reshape(KB, P, N).transpose(1, 0, 2)
    )


_NC_CACHE = {}

# tuned on hardware: nb-outer/kb-inner matmul order, x DMA in 4 chunks/plane
TUNED = dict(swap_loop=True, x_dma_split=4)


def _get_nc(T, K, N, x_split):
    key = (T, K, N, x_split)
    if key not in _NC_CACHE:
        _NC_CACHE[key] = build_nc(T, K, N, x_split=x_split, **TUNED)
    return _NC_CACHE[key]


def kernel(x, w_q, scale, bias):
    x = np.asarray(x)
    w_q = np.asarray(w_q)
    scale = np.asarray(scale, dtype=np.float32)
    bias = np.asarray(bias, dtype=np.float32)
    B, Sq, K = x.shape
    K2, D_OUT = w_q.shape
    assert K2 == K
    T = B * Sq
    N = D_OUT // N_CORES

    nc = _get_nc(T, K, N, X_SPLIT)

    xh = pack_x(np.ascontiguousarray(x.reshape(T, K)), T, K, X_SPLIT)
    in_maps = []
    for c in range(N_CORES):
        sl = slice(c * N, (c + 1) * N)
        in_maps.append(
            {
                "xh": xh,
                "wh": pack_w(w_q[:, sl], K, N),
                "scale": np.ascontiguousarray(scale[sl], dtype=np.float32),
                "bias": np.ascontiguousarray(bias[sl], dtype=np.float32),
            }
        )

    res = run_bass_kernel_spmd(nc, in_maps, core_ids=list(range(N_CORES)))
    y = np.concatenate([r["y"] for r in res.results], axis=1)
    return y.reshape(B, Sq, D_OUT)
